# revision 46
# baseline (speedup 1.0000x reference)
"""Trainium2 Bass kernel: MultiHeadLatentAttention (bf16, batched-DMA, v3).

Problem (hardcoded): B=4, S=1024, HID=2048, NH=16 heads of HD=128, LAT=512,
fp32 interface, causal attention with RoPE, latent-compressed K/V (MLA).

Sharding over 8 NeuronCores: core c = (batch b = c//2, head-group hg = c%2).
Each core handles one batch element and 8 heads (local width HL=1024).

All matmul operands are bf16 (PSUM accumulation stays fp32). Host pre-casts
inputs to bf16. DMAs are batched into multi-k-tile transfers. wo is
host-packed and prefetched during earlier phases. The softmax denominator
is computed OFF the tensor engine: exp tiles are accumulated on DVE/Pool
and reduced across partitions with gpsimd partition_all_reduce; the
1/sqrt(HD) score scale is folded into the Q rope constants on the host.

Phases: QT proj (+rope) -> latT -> KT (+rope) -> V -> attention -> out-proj.
KT runs before V so the rope chains overlap V's matmuls and attention can
start as soon as head 0's K and the first V tiles exist.

Host gathers: out[b] = (f32(outT[2b]) + f32(outT[2b+1])).T + bo.
"""

import os

if "axon" not in os.environ.get("JAX_PLATFORMS", ""):
    os.environ["JAX_PLATFORMS"] = "axon"

import numpy as np

import concourse.bacc as bacc
import concourse.mybir as mybir
import concourse.tile as tile
from concourse import bass_isa
from concourse.bass_utils import run_bass_kernel_spmd

# ---- problem dims (hardcoded per contest rules)
B, S, HID, NH, LAT = 4, 1024, 2048, 16, 512
HD = 128
NHL = NH // 2          # heads per core = 8
HL = NHL * HD          # local head width = 1024
P = 128
KT_H = HID // P        # 16
KT_L = LAT // P        # 4
QCW = 512              # q-chunk width (PSUM bank)
NQC = S // QCW         # 2
SC_SCALE = float(1.0 / np.sqrt(HD))

F32 = mybir.dt.float32
BF16 = mybir.dt.bfloat16

N_CORES = 8


def _rope(nc, pool, raw, out_ap, cos_sb, sin_sb, pool_sin=True,
          dma_shift=False):
    """out(bf16) = raw * cos + shift64(raw) * sinTe, all-bf16 tiles."""
    sh = pool.tile([P, S], BF16, tag="shift", name="sh")
    if dma_shift:  # use DMA when the HBM queues are idle in this phase
        nc.sync.dma_start(sh[0:64, :], raw[64:128, :])
        nc.sync.dma_start(sh[64:128, :], raw[0:64, :])
    else:
        nc.gpsimd.tensor_copy(sh[0:64, :], raw[64:128, :])
        nc.gpsimd.tensor_copy(sh[64:128, :], raw[0:64, :])
    nc.vector.tensor_mul(out_ap, raw, cos_sb)
    if pool_sin:
        nc.gpsimd.tensor_mul(sh, sh, sin_sb)
    else:
        nc.vector.tensor_mul(sh, sh, sin_sb)
    nc.vector.tensor_add(out_ap, out_ap, sh)


def build_bass(loop_iters=None):
    nc = bacc.Bacc("TRN2", target_bir_lowering=False, debug=False, num_devices=8)

    xT = nc.dram_tensor("xT", [HID, S], BF16, kind="ExternalInput")[:]
    wq = nc.dram_tensor("wq", [HID, HL], BF16, kind="ExternalInput")[:]
    wdown = nc.dram_tensor("wdown", [HID, LAT], BF16, kind="ExternalInput")[:]
    wkup = nc.dram_tensor("wkup", [LAT, HL], BF16, kind="ExternalInput")[:]
    wvup = nc.dram_tensor("wvup", [LAT, HL], BF16, kind="ExternalInput")[:]
    # wo host-packed: wop[p, kt*HID + o] = Wo[kt*128 + p, o]
    wop = nc.dram_tensor("wop", [P, NHL * HID], BF16, kind="ExternalInput")[:]
    bqd = nc.dram_tensor("bq", [P, NHL], F32, kind="ExternalInput")[:]
    # cosQ/sinQ carry the 1/sqrt(HD) score scale; sin has rotate-half sign
    cosQd = nc.dram_tensor("cosQ", [P, S], BF16, kind="ExternalInput")[:]
    sinQd = nc.dram_tensor("sinQ", [P, S], BF16, kind="ExternalInput")[:]
    cosKd = nc.dram_tensor("cosK", [P, S], BF16, kind="ExternalInput")[:]
    sinKd = nc.dram_tensor("sinK", [P, S], BF16, kind="ExternalInput")[:]
    maskTd = nc.dram_tensor("maskT", [P, 3 * P], BF16, kind="ExternalInput")[:]
    onesqd = nc.dram_tensor("ones_sq", [P, P], BF16, kind="ExternalInput")[:]
    outT = nc.dram_tensor("outT", [HID, S], BF16, kind="ExternalOutput")[:]

    import contextlib

    with tile.TileContext(nc) as tc, contextlib.ExitStack() as _les:
        if loop_iters is not None:
            # hint the back-edge branch target for the big-body engines
            # (IRAM I$-miss costs ~3-4us/iteration otherwise) and reset
            # semaphores without the two all-engine barriers
            _les.enter_context(tc.For_i(
                0, loop_iters, 1,
                hint_engines=(mybir.EngineType.PE,
                              mybir.EngineType.Activation),
                staggered_reset=True,
            ))
        with (
            tc.tile_pool(name="consts", bufs=1) as consts,
            tc.tile_pool(name="resident", bufs=1) as resident,
            tc.tile_pool(name="psc", bufs=2, space="PSUM") as psc,
        ):
            cosQ_sb = consts.tile([P, S], BF16)
            sinQ_sb = consts.tile([P, S], BF16)
            cosK_sb = consts.tile([P, S], BF16)
            sinK_sb = consts.tile([P, S], BF16)
            mask_sb = consts.tile([P, 3 * P], BF16)
            bq_sb = consts.tile([P, NHL], F32)
            ones_sq = consts.tile([P, P], BF16)

            qT_sb = resident.tile([P, NHL, S], BF16)
            kT_sb = resident.tile([P, NHL, S], BF16)
            wo_sb = resident.tile([P, NHL, HID], BF16)

            # V outlives latT/pacc (released LIFO), so its pool opens first
            vpool_cm = tc.tile_pool(name="vres", bufs=1)
            vpool = vpool_cm.__enter__()
            v_sb = vpool.tile([P, NHL, HL], BF16)  # [s%128, s//128, hl]

            # K/V up-projection weights live at top level so their DMAs can
            # be issued during phase 1 (SP executes triggers in program
            # order; issuing them late stalls the KT phase ~5us)
            wkv_cm = tc.tile_pool(name="wkv", bufs=1)
            wkv = wkv_cm.__enter__()
            wtk = wkv.tile([P, KT_L, HL], BF16)
            wtv = wkv.tile([P, KT_L, HL], BF16)

            # PSUM accumulators for all projection phases (6 banks; psc has 2)
            pacc_cm = tc.tile_pool(name="pacc", bufs=6, space="PSUM")
            pacc = pacc_cm.__enter__()

            latp_cm = tc.tile_pool(name="latp", bufs=1)
            latp = latp_cm.__enter__()
            latT_sb = latp.tile([P, KT_L, S], BF16)

            def alloc8():
                """8 psum accumulators: 6 from pacc + 2 borrowed from psc."""
                ps = {}
                for i in range(8):
                    oi, ntc = divmod(i, NQC)
                    pool = pacc if i < 6 else psc
                    tag = "acc" if pool is pacc else "sc"
                    ps[(oi, ntc)] = pool.tile([P, QCW], F32, tag=tag, name="acc")
                return ps

            # ---------- phases 1-2: QT (bias+rope), latT
            with (
                tc.tile_pool(name="xp", bufs=1) as xp,
                tc.tile_pool(name="ws1", bufs=3) as ws1,
                tc.tile_pool(name="rope1", bufs=3) as rp1,
            ):
                xT_sb = xp.tile([P, KT_H, S], BF16)
                nc.sync.dma_start(bq_sb, bqd)

                def qt_rope(h, ps_oi_ntc):
                    raw = rp1.tile([P, S], BF16, tag="raw", name="raw")
                    for ntc in range(NQC):
                        nc.scalar.add(
                            raw[:, ntc * QCW:(ntc + 1) * QCW],
                            ps_oi_ntc[ntc],
                            bq_sb[:, h:h + 1],
                        )
                    _rope(nc, rp1, raw, qT_sb[:, h, :], cosQ_sb, sinQ_sb,
                          pool_sin=(h % 2 == 0))

                # QT in two groups of 4 heads; xT streamed in 4-ktile chunks
                # during group 0; weights in 4-ktile chunks throughout.
                # Order: QT og0 -> latT -> QT og1, so latT's PSUM drain (which
                # gates the KT phase) hides under og1's 27us of matmuls.
                xTr = xT.rearrange("(k p) s -> p k s", p=P)

                def qt_group(og):
                    ps = alloc8()
                    wts = {}
                    for kt in range(KT_H):
                        kg, ko = divmod(kt, 4)
                        if kt % 4 == 0:
                            if og == 0:
                                if kg == 0:
                                    # first chunk split small so the first
                                    # matmuls start ~4us earlier; issue the
                                    # two deps of matmul 0 before the rest
                                    nc.sync.dma_start(
                                        xT_sb[:, 0:1, :], xTr[:, 0:1, :])
                                    wqr0 = wq.rearrange(
                                        "(k p) o -> p k o", p=P
                                    )[:, 0:4, 0:QCW]
                                    wt0 = ws1.tile([P, 4, QCW], BF16,
                                                   tag="w4", name="wt")
                                    nc.sync.dma_start(
                                        wt0[:, 0:1, :], wqr0[:, 0:1, :])
                                    nc.sync.dma_start(
                                        xT_sb[:, 1:4, :], xTr[:, 1:4, :])
                                    nc.sync.dma_start(
                                        wt0[:, 1:4, :], wqr0[:, 1:4, :])
                                    wts[0] = wt0
                                else:
                                    nc.sync.dma_start(
                                        xT_sb[:, kg * 4:(kg + 1) * 4, :],
                                        xTr[:, kg * 4:(kg + 1) * 4, :],
                                    )
                            if not (og == 0 and kg == 0):
                                wt = ws1.tile([P, 4, QCW], BF16, tag="w4",
                                              name="wt")
                                nc.sync.dma_start(
                                    wt,
                                    wq.rearrange("(k p) o -> p k o", p=P)[
                                        :, kg * 4:(kg + 1) * 4,
                                        og * QCW:(og + 1) * QCW],
                                )
                                wts[kg] = wt
                        for oi in range(4):
                            for ntc in range(NQC):
                                nc.tensor.matmul(
                                    ps[(oi, ntc)],
                                    lhsT=wts[kg][:, ko, oi * P:(oi + 1) * P],
                                    rhs=xT_sb[:, kt, ntc * QCW:(ntc + 1) * QCW],
                                    start=(kt == 0),
                                    stop=(kt == KT_H - 1),
                                )
                    if og == 0:
                        nc.sync.dma_start(cosQ_sb, cosQd)
                        nc.sync.dma_start(sinQ_sb, sinQd)
                        nc.sync.dma_start(cosK_sb, cosKd)
                        nc.sync.dma_start(sinK_sb, sinKd)
                        # prefetch wkup behind og0's stream; KT needs it
                        # ~55us from here
                        nc.sync.dma_start(
                            wtk, wkup.rearrange("(k p) o -> p k o", p=P)
                        )
                    else:
                        # og1's queue window: prefetch wvup + the whole
                        # out-projection weight (needed from ~95us / ~125us)
                        nc.sync.dma_start(
                            wtv, wvup.rearrange("(k p) o -> p k o", p=P)
                        )
                        woup = wop.rearrange("p (k o) -> p k o", k=NHL)
                        nc.sync.dma_start(wo_sb[:, 0:4, :], woup[:, 0:4, :])
                        nc.sync.dma_start(wo_sb[:, 4:8, :], woup[:, 4:8, :])
                    for oi in range(4):
                        h = og * 4 + oi
                        qt_rope(h, {ntc: ps[(oi, ntc)] for ntc in range(NQC)})

                def lat_group():
                    ps = alloc8()
                    wts = {}
                    nc.sync.dma_start(mask_sb, maskTd)
                    nc.sync.dma_start(ones_sq, onesqd)
                    for kt in range(KT_H):
                        kg, ko = divmod(kt, 4)
                        if kt % 4 == 0:
                            wt = ws1.tile([P, 4, QCW], BF16, tag="w4", name="wt")
                            nc.sync.dma_start(
                                wt,
                                wdown.rearrange("(k p) o -> p k o", p=P)[
                                    :, kg * 4:(kg + 1) * 4, :],
                            )
                            wts[kg] = wt
                        for oi in range(4):
                            for ntc in range(NQC):
                                nc.tensor.matmul(
                                    ps[(oi, ntc)],
                                    lhsT=wts[kg][:, ko, oi * P:(oi + 1) * P],
                                    rhs=xT_sb[:, kt, ntc * QCW:(ntc + 1) * QCW],
                                    start=(kt == 0),
                                    stop=(kt == KT_H - 1),
                                )
                    for oi in range(4):
                        for ntc in range(NQC):
                            # split the drain across ACT/DVE so KT's matmuls
                            # aren't gated on a serial ACT copy chain
                            dst = latT_sb[:, oi, ntc * QCW:(ntc + 1) * QCW]
                            if ntc == 0:
                                nc.scalar.copy(dst, ps[(oi, ntc)])
                            else:
                                nc.vector.tensor_copy(dst, ps[(oi, ntc)])

                qt_group(0)
                qt_group(1)
                lat_group()

            # xT / ws1 / rope1 freed here

            # ---------- phase 3: KT (rope per head, contract latT over LAT)
            # Runs BEFORE V so the rope chains overlap V's matmuls.
            with (
                tc.tile_pool(name="rope3", bufs=3) as rp3,
            ):
                for og in range(4):  # 2 heads per group, pacc only
                    ps = {}
                    for oi in range(2):
                        for ntc in range(NQC):
                            ps[(oi, ntc)] = pacc.tile(
                                [P, QCW], F32, tag="acc", name="acc"
                            )
                    for kt in range(KT_L):
                        for oi in range(2):
                            for ntc in range(NQC):
                                nc.tensor.matmul(
                                    ps[(oi, ntc)],
                                    lhsT=wtk[:, kt,
                                             (og * 2 + oi) * P:
                                             (og * 2 + oi + 1) * P],
                                    rhs=latT_sb[:, kt,
                                                ntc * QCW:(ntc + 1) * QCW],
                                    start=(kt == 0),
                                    stop=(kt == KT_L - 1),
                                )
                    for oi in range(2):
                        h = og * 2 + oi
                        raw = rp3.tile([P, S], BF16, tag="raw", name="raw")
                        for ntc in range(NQC):
                            nc.scalar.copy(
                                raw[:, ntc * QCW:(ntc + 1) * QCW], ps[(oi, ntc)]
                            )
                        _rope(nc, rp3, raw, kT_sb[:, h, :], cosK_sb, sinK_sb,
                              pool_sin=(oi % 2 == 0), dma_shift=True)

                # ---------- phase 4: V natural [S, HL]
                for hlc in range(2):
                    for sg in range(2):
                        ps = {}
                        for si in range(4):
                            ps[si] = pacc.tile([P, QCW], F32, tag="acc",
                                               name="acc")
                        for kt in range(KT_L):
                            for si in range(4):
                                st = sg * 4 + si
                                nc.tensor.matmul(
                                    ps[si],
                                    lhsT=latT_sb[:, kt, st * P:(st + 1) * P],
                                    rhs=wtv[:, kt,
                                            hlc * QCW:(hlc + 1) * QCW],
                                    start=(kt == 0),
                                    stop=(kt == KT_L - 1),
                                )
                        for si in range(4):
                            st = sg * 4 + si
                            nc.scalar.copy(
                                v_sb[:, st, hlc * QCW:(hlc + 1) * QCW], ps[si]
                            )

            latp_cm.__exit__(None, None, None)  # free latT zone
            pacc_cm.__exit__(None, None, None)  # free PSUM for attention pools
            wkv_cm.__exit__(None, None, None)   # free K/V weight zone

            # ---------- attention + output projection
            with (
                tc.tile_pool(name="ctxp", bufs=1) as ctxp,
                tc.tile_pool(name="exl", bufs=8) as exl,
                tc.tile_pool(name="small", bufs=3) as small,
                tc.tile_pool(name="pctx", bufs=2, space="PSUM") as pctx,
                tc.tile_pool(name="psum1", bufs=2, space="PSUM") as psum1,
                tc.tile_pool(name="pbcpo", bufs=2, space="PSUM") as pbcpo,
                tc.tile_pool(name="outsb", bufs=3) as outsb,
            ):
                ctxT_sb = ctxp.tile([P, NHL, S], BF16)

                def finalize(fin):
                    # sums is already broadcast across partitions (the ones
                    # stationary is [P,P]), so normalize straight from PSUM.
                    ctx_f, sums_f, h_f, qc_f = fin
                    rec = small.tile([P, QCW], F32, tag="rec", name="rec")
                    nc.vector.reciprocal(out=rec, in_=sums_f)
                    nc.vector.tensor_mul(
                        ctxT_sb[:, h_f, qc_f * QCW:(qc_f + 1) * QCW], ctx_f, rec
                    )

                def out_proj(ot, qc):
                    po = pbcpo.tile([P, QCW], F32, tag="bcpo", name="po")
                    for kt in range(NHL):
                        nc.tensor.matmul(
                            po,
                            lhsT=wo_sb[:, kt, ot * P:(ot + 1) * P],
                            rhs=ctxT_sb[:, kt, qc * QCW:(qc + 1) * QCW],
                            start=(kt == 0),
                            stop=(kt == NHL - 1),
                        )
                    osb = outsb.tile([P, QCW], BF16, tag="osb")
                    nc.any.tensor_copy(osb, po)
                    nc.sync.dma_start(
                        outT[ot * P:(ot + 1) * P, qc * QCW:(qc + 1) * QCW],
                        osb,
                    )

                pending = None
                for qc in range(NQC):
                    for h in range(NHL):
                        nkt = 4 * qc + 4  # k-tiles covering causal range
                        ctx = pctx.tile([P, QCW], F32, tag="ctx")
                        acc = small.tile([P, QCW], BF16, tag="eacc", name="acc")

                        def block_geom(kt):
                            """(lo, w, mask_ap): sliced q-range for causal."""
                            off = kt - 4 * qc
                            if off < 0:
                                return 0, QCW, None
                            if off == 0:
                                return 0, QCW, mask_sb[:, 0:P]
                            if off < 3:
                                return 128 * off, QCW - 128 * off, \
                                    mask_sb[:, 0:P]
                            # off == 3: only the 128-wide triangle survives
                            # (bf16 matmuls have no >=256 moving constraint)
                            return 384, 128, mask_sb[:, 0:P]

                        def emit_sc(kt):
                            lo, w, mk = block_geom(kt)
                            sc = psc.tile([P, QCW], F32, tag="sc", name="sc")
                            nc.tensor.matmul(
                                sc[:, :w],
                                lhsT=kT_sb[:, h, kt * P:(kt + 1) * P],
                                rhs=qT_sb[:, h,
                                          qc * QCW + lo:qc * QCW + lo + w],
                                start=True,
                                stop=True,
                            )
                            ex = exl.tile([P, QCW], BF16, tag="ex", name="ex")
                            nc.scalar.activation(
                                out=ex[:, :w], in_=sc[:, :w],
                                func=mybir.ActivationFunctionType.Exp,
                            )
                            if mk is not None:  # causal zeroing of the triangle
                                mw = mk.shape[-1]
                                eng = nc.vector if (kt % 2) else nc.gpsimd
                                eng.tensor_mul(ex[:, :mw], ex[:, :mw], mk)
                            return ex

                        def emit_pv(kt, ex):
                            lo, w, _ = block_geom(kt)
                            nc.tensor.matmul(
                                ctx[:, lo:lo + w],
                                lhsT=v_sb[:, kt, h * P:(h + 1) * P],
                                rhs=ex[:, :w],
                                start=(kt == 0),
                                stop=(kt == nkt - 1),
                            )
                            # pre-sum exp tiles on DVE/Pool; one ones-matmul
                            # per (h, qc) replaces the per-kt PE reduction
                            eng = nc.gpsimd if (kt % 2) else nc.vector
                            if kt == 0:
                                eng.tensor_copy(acc, ex)
                            else:
                                eng.tensor_add(
                                    acc[:, lo:], acc[:, lo:], ex[:, :w]
                                )

                        # software-pipelined emission: sc(kt+1) before pv(kt)
                        exs = {0: emit_sc(0)}
                        for kt in range(nkt):
                            if kt + 1 < nkt:
                                exs[kt + 1] = emit_sc(kt + 1)
                            emit_pv(kt, exs.pop(kt))
                        sums = psum1.tile([P, QCW], F32, tag="sums")
                        nc.tensor.matmul(
                            sums, lhsT=ones_sq, rhs=acc, start=True, stop=True
                        )
                        # normalize the PREVIOUS head so its reciprocal has a
                        # whole head of PE work to hide behind
                        if pending is not None:
                            finalize(pending)
                        pending = (ctx, sums, h, qc)
                        # interleave qc0's out-projection into qc1's
                        # (ACT-bound) attention to keep the PE fed
                        if qc == 1:
                            out_proj(2 * h, 0)
                            out_proj(2 * h + 1, 0)
                finalize(pending)

                # remaining out-projection: qc1 columns
                for ot in range(HID // P):
                    out_proj(ot, 1)
            vpool_cm.__exit__(None, None, None)
    nc.compile()
    return nc


# ---------------- host side ----------------

def _host_consts():
    inv_freq = 1.0 / (10000.0 ** (np.arange(0, HD, 2, dtype=np.float64) / HD))
    t = np.arange(S, dtype=np.float64)
    freqs = t[:, None] * inv_freq[None, :]            # [S, 64]
    emb = np.concatenate([freqs, freqs], axis=-1)     # [S, 128]
    cosT = np.cos(emb).T.astype(np.float32).copy()    # [128, S]
    sinT = np.sin(emb).T.astype(np.float32)
    sinTe = sinT.copy()
    sinTe[:64] *= -1.0                                # sign of rotate_half folded in
    sinTe = np.ascontiguousarray(sinTe.astype(np.float32))

    ii = np.arange(P)[:, None]
    tri = (np.arange(P)[None, :] - ii >= 0).astype(np.float32)       # [128,128]
    maskb = np.concatenate([np.zeros((P, P), np.float32), tri], axis=1)
    maskT = np.ascontiguousarray(np.concatenate([tri, maskb], axis=1))  # [128,384]
    return cosT, sinTe, maskT


_CACHE = {}


def _get_built():
    if "nc" not in _CACHE:
        _CACHE["nc"] = build_bass()
        _CACHE["consts"] = _host_consts()
    return _CACHE["nc"], _CACHE["consts"]


def _bf16(a):
    import ml_dtypes
    return np.ascontiguousarray(np.asarray(a).astype(ml_dtypes.bfloat16))


def make_in_maps(x, Wq, bq, Wdown, Wk_up, Wv_up, Wo):
    cosT, sinTe, maskT = _get_built()[1]
    import ml_dtypes
    bf = ml_dtypes.bfloat16
    in_maps = []
    for c in range(N_CORES):
        b, hg = c // 2, c % 2
        sl = slice(hg * HL, (hg + 1) * HL)
        wo_s = np.asarray(Wo[sl, :])                      # [HL, HID]
        wo_p = wo_s.reshape(NHL, P, HID).transpose(1, 0, 2).reshape(P, NHL * HID)
        in_maps.append({
            "xT": _bf16(x[b].T),
            "wq": _bf16(Wq[:, sl]),
            "wdown": _bf16(Wdown),
            "wkup": _bf16(Wk_up[:, sl]),
            "wvup": _bf16(Wv_up[:, sl]),
            "wop": _bf16(wo_p),
            "bq": np.ascontiguousarray(bq[sl].reshape(NHL, P).T),
            "cosQ": _bf16(cosT * SC_SCALE),
            "sinQ": _bf16(sinTe * SC_SCALE),
            "cosK": _bf16(cosT),
            "sinK": _bf16(sinTe),
            "maskT": maskT.astype(bf),
            "ones_sq": np.ones((P, P), bf),
        })
    return in_maps


def gather_out(results, bo):
    out = np.empty((B, S, HID), dtype=np.float32)
    for b in range(B):
        acc = results[2 * b]["outT"].astype(np.float32) \
            + results[2 * b + 1]["outT"].astype(np.float32)  # [HID, S]
        out[b] = acc.T + bo[None, :]
    return out


def kernel(x, Wq, bq, Wdown, Wk_up, Wv_up, Wo, bo):
    x = np.asarray(x, dtype=np.float32)
    Wq = np.asarray(Wq, dtype=np.float32)
    bq = np.asarray(bq, dtype=np.float32)
    Wdown = np.asarray(Wdown, dtype=np.float32)
    Wk_up = np.asarray(Wk_up, dtype=np.float32)
    Wv_up = np.asarray(Wv_up, dtype=np.float32)
    Wo = np.asarray(Wo, dtype=np.float32)
    bo = np.asarray(bo, dtype=np.float32)

    nc, _ = _get_built()
    in_maps = make_in_maps(x, Wq, bq, Wdown, Wk_up, Wv_up, Wo)
    res = run_bass_kernel_spmd(nc, in_maps, core_ids=list(range(N_CORES)))
    return gather_out(res.results, bo)
